# revision 7
# baseline (speedup 1.0000x reference)
"""Trainium2 Bass kernel for gather + segment_sum (segment_reduce).

reference:
    gathered = raw_h[member_idx]                      # [N_MEMBERS, HIDDEN]
    out = segment_sum(gathered, segment_ids, num_segments=N_CLIQUES)

Strategy (8-core SPMD, data-parallel over cliques):
  - Shard cliques contiguously: core c owns cliques [c*50000, (c+1)*50000).
    segment_ids is sorted, so each core's members are a contiguous slice
    (host computes boundaries with searchsorted). No cross-core traffic.
  - Per core, cliques are processed in windows of 128 segments.  Each
    window's member list is padded to J*128 slots (J = max needed over all
    windows/cores, data-derived), giving a fully static SPMD program:
    window t <-> member tiles [J*t, J*t+J).
  - Each member tile (128 members) is gathered from raw_h with one
    indirect DMA (one row per SBUF partition, int32 row indices).
  - Segment-sum within a window: one-hot selection matrices built on-device
    (is_equal against an iota matrix) and accumulated on the TensorEngine:
    psum[seg, :] += onehot[member, seg]^T @ gathered[member, :].
  - PSUM -> SBUF staging -> plain DMA to the output shard.
Pad slots gather row 0 (finite, harmless) and carry seg = -1 so their
one-hot column is all zero; they contribute nothing.
"""

import os
import numpy as np

N_ATOMS = 500000
N_MEMBERS = 1500000
N_CLIQUES = 400000
HIDDEN = 128
N_CORES = 8
CPC = N_CLIQUES // N_CORES          # cliques per core
WIN = 128                            # segments per window (= PSUM tile rows)
T_OUT = (CPC + WIN - 1) // WIN       # windows per core (391)
OUT_ROWS = T_OUT * WIN               # padded output rows per core (50048)
G_STAGE = 8                          # windows per output staging DMA

LAST_EXEC_NS = None                  # filled when BASS_TRACE=1


def _install_ntff_hook():
    """Register the axon NTFF profile hook if available (for BASS_TRACE)."""
    try:
        import sys, types
        import trn_agent_boot.trn_boot as tb
        hook = tb._ntff_profile_via_ctypes('/opt/axon/libaxon_pjrt.so')
        if hook is None:
            return
        mod = types.ModuleType('antenv.axon_hooks')
        mod.get_axon_ntff_profile_hook = lambda: hook
        mod.set_axon_ntff_profile_hook = lambda h: None
        import antenv
        sys.modules['antenv.axon_hooks'] = mod
        antenv.axon_hooks = mod
        from concourse import bass_utils
        bass_utils.upload_artifacts = lambda d: f"local:{d}"
    except Exception:
        pass


def _host_prep(member_idx, segment_ids):
    """Build per-core padded index/segment arrays. Returns (J, idx_t, seg_t)."""
    seg = np.asarray(segment_ids).astype(np.int64)
    atom = np.asarray(member_idx).astype(np.int64)
    bounds = np.searchsorted(seg, np.arange(0, N_CLIQUES + 1, CPC))

    cores = []
    J = 1
    for c in range(N_CORES):
        lo, hi = int(bounds[c]), int(bounds[c + 1])
        seg_local = (seg[lo:hi] - c * CPC).astype(np.int64)
        atoms_c = atom[lo:hi].astype(np.int32)
        w = np.searchsorted(seg_local, np.arange(0, OUT_ROWS + 1, WIN))
        counts = np.diff(w)
        if counts.max() > 0:
            J = max(J, int(-(-counts.max() // 128)))
        cores.append((seg_local, atoms_c, w))

    NT = T_OUT * J
    idx_list, seg_list = [], []
    for seg_local, atoms_c, w in cores:
        n = len(seg_local)
        t_of_m = (seg_local >> 7).astype(np.int64)          # window of member
        s_in_w = np.arange(n, dtype=np.int64) - w[t_of_m]   # slot within window
        col = t_of_m * J + (s_in_w >> 7)
        row = s_in_w & 127
        idx_t = np.full((128, NT), N_ATOMS, np.int32)       # pad: OOB -> skipped
        seg_t = np.full((128, NT), -1.0, np.float32)        # pad: seg -1
        idx_t[row, col] = atoms_c
        seg_t[row, col] = (seg_local - (t_of_m << 7)).astype(np.float32)
        idx_list.append(idx_t)
        seg_list.append(seg_t)
    return J, idx_list, seg_list


def _build_program(J):
    from concourse import bass, bacc, mybir
    import concourse.tile as tile

    NT = T_OUT * J
    NBUF = 8
    NQ = 4   # spread indirect DMAs across all SWDGE queues (parallel desc-gen)

    nc = bacc.Bacc("TRN2", target_bir_lowering=False, debug=False,
                   num_devices=N_CORES, num_swdge_queues=NQ)
    raw_p = nc.declare_dram_parameter("raw_h", [N_ATOMS, HIDDEN],
                                      mybir.dt.float32, isOutput=False)
    idx_p = nc.declare_dram_parameter("idx_t", [128, NT],
                                      mybir.dt.int32, isOutput=False)
    seg_p = nc.declare_dram_parameter("seg_t", [128, NT],
                                      mybir.dt.float32, isOutput=False)
    out_p = nc.declare_dram_parameter("out", [OUT_ROWS, HIDDEN],
                                      mybir.dt.float32, isOutput=True)

    with tile.TileContext(nc) as tc:
        with tc.tile_pool(name="const", bufs=1) as cpool, \
             tc.tile_pool(name="gb", bufs=1) as gpool, \
             tc.tile_pool(name="oh", bufs=6) as opool, \
             tc.tile_pool(name="stage", bufs=3) as spool, \
             tc.tile_pool(name="psum", bufs=6, space="PSUM") as ppool:

            idx_sb = cpool.tile([128, NT], mybir.dt.int32)
            seg_sb = cpool.tile([128, NT], mybir.dt.float32)
            iota = cpool.tile([128, J * 128], mybir.dt.float32)
            nc.sync.dma_start(out=idx_sb[:], in_=idx_p[:])
            nc.sync.dma_start(out=seg_sb[:], in_=seg_p[:])
            # iota[p, j*128+m] = m   (same for every partition / column group)
            nc.gpsimd.iota(iota[:].rearrange("p (j m) -> p j m", j=J),
                           pattern=[[0, J], [1, 128]], base=0,
                           channel_multiplier=0,
                           allow_small_or_imprecise_dtypes=True)

            gbs = [gpool.tile([128, J * 128], mybir.dt.float32, name=f"gb{b}")
                   for b in range(NBUF)]
            # one-time zero so OOB-skipped pad slots read finite stale data
            for b in range(NBUF):
                nc.vector.memset(gbs[b][:], 0.0)

            n_groups = (T_OUT + G_STAGE - 1) // G_STAGE
            for grp in range(n_groups):
                t0 = grp * G_STAGE
                used = min(G_STAGE, T_OUT - t0)
                stg = spool.tile([128, G_STAGE * 128], mybir.dt.float32,
                                 name="stg")
                for tt in range(used):
                    t = t0 + tt
                    gb = gbs[t % NBUF]
                    # gather the window's J member tiles
                    for j in range(J):
                        col = t * J + j
                        h = nc.gpsimd.indirect_dma_start(
                            out=gb[:, j * 128:(j + 1) * 128],
                            out_offset=None,
                            in_=raw_p[:],
                            in_offset=bass.IndirectOffsetOnAxis(
                                ap=idx_sb[:, col:col + 1], axis=0),
                            bounds_check=N_ATOMS - 1,
                            oob_is_err=False,
                        )
                        q = col % NQ
                        if q:
                            h.ins.queue = f"qPoolDynamic{q}"
                    # one-hot for the whole window in one compare
                    oh = opool.tile([128, J * 128], mybir.dt.float32,
                                    name="oh")
                    nc.any.tensor_tensor(
                        out=oh[:].rearrange("p (j m) -> p j m", j=J),
                        in0=seg_sb[:, t * J:(t + 1) * J].to_broadcast(
                            [128, J, 128]),
                        in1=iota[:].rearrange("p (j m) -> p j m", j=J),
                        op=mybir.AluOpType.is_equal,
                    )
                    ps = ppool.tile([128, 128], mybir.dt.float32,
                                    space="PSUM", name="ps")
                    for j in range(J):
                        nc.tensor.matmul(
                            out=ps[:],
                            lhsT=oh[:, j * 128:(j + 1) * 128],
                            rhs=gb[:, j * 128:(j + 1) * 128],
                            start=(j == 0),
                            stop=(j == J - 1),
                        )
                    nc.any.tensor_copy(out=stg[:, tt * 128:(tt + 1) * 128],
                                       in_=ps[:])
                # staging -> DRAM: row (g*128 + p) <- stg[p, g*128:]
                out_view = out_p[t0 * 128:(t0 + used) * 128, :].rearrange(
                    "(g p) d -> p g d", p=128)
                nc.sync.dma_start(
                    out=out_view,
                    in_=stg[:, :used * 128].rearrange("p (g d) -> p g d",
                                                      d=128))
    nc.compile()
    return nc


def kernel(raw_h, member_idx, segment_ids):
    global LAST_EXEC_NS
    from concourse.bass_utils import run_bass_kernel_spmd

    if os.environ.get("BASS_TRACE"):
        _install_ntff_hook()

    raw = np.ascontiguousarray(np.asarray(raw_h, dtype=np.float32))
    J, idx_list, seg_list = _host_prep(member_idx, segment_ids)
    nc = _build_program(J)

    in_maps = [
        {"raw_h": raw, "idx_t": idx_list[c], "seg_t": seg_list[c]}
        for c in range(N_CORES)
    ]
    res = run_bass_kernel_spmd(nc, in_maps, core_ids=list(range(N_CORES)))
    LAST_EXEC_NS = res.exec_time_ns

    out = np.concatenate(
        [res.results[c]["out"][:CPC] for c in range(N_CORES)], axis=0)
    return out.astype(np.float32)


# revision 10
# speedup vs baseline: 1.4497x; 1.4497x over previous
"""Trainium2 Bass kernel for gather + segment_sum (segment_reduce).

reference:
    gathered = raw_h[member_idx]                      # [N_MEMBERS, HIDDEN]
    out = segment_sum(gathered, segment_ids, num_segments=N_CLIQUES)

Strategy (8-core SPMD, data-parallel over cliques):
  - Shard cliques contiguously: core c owns cliques [c*50000, (c+1)*50000).
    segment_ids is sorted, so each core's members are a contiguous slice
    (host computes boundaries with searchsorted). No cross-core traffic.
  - Per core, cliques are processed in windows of 128 segments.  Each
    window's member list is padded to J*128 slots (J = max needed over all
    windows/cores, data-derived), giving a fully static SPMD program:
    window t <-> member tiles [J*t, J*t+J).
  - Each member tile (128 members) is gathered from raw_h with one
    indirect DMA (one row per SBUF partition, int32 row indices).
  - Segment-sum within a window: one-hot selection matrices built on-device
    (is_equal against an iota matrix) and accumulated on the TensorEngine:
    psum[seg, :] += onehot[member, seg]^T @ gathered[member, :].
  - PSUM -> SBUF staging -> plain DMA to the output shard.
Pad slots gather row 0 (finite, harmless) and carry seg = -1 so their
one-hot column is all zero; they contribute nothing.
"""

import os
import numpy as np

N_ATOMS = 500000
N_MEMBERS = 1500000
N_CLIQUES = 400000
HIDDEN = 128
N_CORES = 8
CPC = N_CLIQUES // N_CORES          # cliques per core
WIN = 128                            # segments per window (= PSUM tile rows)
T_OUT = (CPC + WIN - 1) // WIN       # windows per core (391)
OUT_ROWS = T_OUT * WIN               # padded output rows per core (50048)
G_STAGE = 8                          # windows per output staging DMA

LAST_EXEC_NS = None                  # filled when BASS_TRACE=1


def _install_ntff_hook():
    """Register the axon NTFF profile hook if available (for BASS_TRACE)."""
    try:
        import sys, types
        import trn_agent_boot.trn_boot as tb
        hook = tb._ntff_profile_via_ctypes('/opt/axon/libaxon_pjrt.so')
        if hook is None:
            return
        mod = types.ModuleType('antenv.axon_hooks')
        mod.get_axon_ntff_profile_hook = lambda: hook
        mod.set_axon_ntff_profile_hook = lambda h: None
        import antenv
        sys.modules['antenv.axon_hooks'] = mod
        antenv.axon_hooks = mod
        from concourse import bass_utils
        bass_utils.upload_artifacts = lambda d: f"local:{d}"
    except Exception:
        pass


def _host_prep(member_idx, segment_ids):
    """Build per-core padded index/segment arrays. Returns (J, idx_t, seg_t)."""
    seg = np.asarray(segment_ids).astype(np.int64)
    atom = np.asarray(member_idx).astype(np.int64)
    bounds = np.searchsorted(seg, np.arange(0, N_CLIQUES + 1, CPC))

    cores = []
    J = 1
    for c in range(N_CORES):
        lo, hi = int(bounds[c]), int(bounds[c + 1])
        seg_local = (seg[lo:hi] - c * CPC).astype(np.int64)
        atoms_c = atom[lo:hi].astype(np.int32)
        w = np.searchsorted(seg_local, np.arange(0, OUT_ROWS + 1, WIN))
        counts = np.diff(w)
        if counts.max() > 0:
            J = max(J, int(-(-counts.max() // 128)))
        cores.append((seg_local, atoms_c, w))

    NT = T_OUT * J
    idx_list, seg_list = [], []
    for seg_local, atoms_c, w in cores:
        n = len(seg_local)
        t_of_m = (seg_local >> 7).astype(np.int64)          # window of member
        s_in_w = np.arange(n, dtype=np.int64) - w[t_of_m]   # slot within window
        col = t_of_m * J + (s_in_w >> 7)
        row = s_in_w & 127
        idx_t = np.zeros((128, NT), np.int32)               # pad: atom 0
        seg_t = np.full((128, NT), -1.0, np.float32)        # pad: seg -1
        idx_t[row, col] = atoms_c
        seg_t[row, col] = (seg_local - (t_of_m << 7)).astype(np.float32)
        idx_list.append(idx_t)
        seg_list.append(seg_t)
    return J, idx_list, seg_list


def _build_program(J):
    from concourse import bass, bacc, mybir
    import concourse.tile as tile

    NT = T_OUT * J
    NBUF = 8
    NQ = 4   # spread indirect DMAs across all SWDGE queues (parallel desc-gen)

    nc = bacc.Bacc("TRN2", target_bir_lowering=False, debug=False,
                   num_devices=N_CORES, num_swdge_queues=NQ)
    raw_p = nc.declare_dram_parameter("raw_h", [N_ATOMS, HIDDEN],
                                      mybir.dt.float32, isOutput=False)
    idx_p = nc.declare_dram_parameter("idx_t", [128, NT],
                                      mybir.dt.int32, isOutput=False)
    seg_p = nc.declare_dram_parameter("seg_t", [128, NT],
                                      mybir.dt.float32, isOutput=False)
    out_p = nc.declare_dram_parameter("out", [OUT_ROWS, HIDDEN],
                                      mybir.dt.float32, isOutput=True)

    with tile.TileContext(nc) as tc:
        with tc.tile_pool(name="const", bufs=1) as cpool, \
             tc.tile_pool(name="gb", bufs=1) as gpool, \
             tc.tile_pool(name="oh", bufs=6) as opool, \
             tc.tile_pool(name="stage", bufs=3) as spool, \
             tc.tile_pool(name="psum", bufs=6, space="PSUM") as ppool:

            idx_sb = cpool.tile([128, NT], mybir.dt.int32)
            seg_sb = cpool.tile([128, NT], mybir.dt.float32)
            iota = cpool.tile([128, J * 128], mybir.dt.float32)
            nc.sync.dma_start(out=idx_sb[:], in_=idx_p[:])
            nc.sync.dma_start(out=seg_sb[:], in_=seg_p[:])
            # iota[p, j*128+m] = m   (same for every partition / column group)
            nc.gpsimd.iota(iota[:].rearrange("p (j m) -> p j m", j=J),
                           pattern=[[0, J], [1, 128]], base=0,
                           channel_multiplier=0,
                           allow_small_or_imprecise_dtypes=True)

            gbs = [gpool.tile([128, J * 128], mybir.dt.float32, name=f"gb{b}")
                   for b in range(NBUF)]

            n_groups = (T_OUT + G_STAGE - 1) // G_STAGE
            for grp in range(n_groups):
                t0 = grp * G_STAGE
                used = min(G_STAGE, T_OUT - t0)
                stg = spool.tile([128, G_STAGE * 128], mybir.dt.float32,
                                 name="stg")
                for tt in range(used):
                    t = t0 + tt
                    gb = gbs[t % NBUF]
                    # gather the window's J member tiles
                    for j in range(J):
                        col = t * J + j
                        h = nc.gpsimd.indirect_dma_start(
                            out=gb[:, j * 128:(j + 1) * 128],
                            out_offset=None,
                            in_=raw_p[:],
                            in_offset=bass.IndirectOffsetOnAxis(
                                ap=idx_sb[:, col:col + 1], axis=0),
                        )
                        q = col % NQ
                        if q:
                            h.ins.queue = f"qPoolDynamic{q}"
                    # one-hot for the whole window in one compare
                    oh = opool.tile([128, J * 128], mybir.dt.float32,
                                    name="oh")
                    nc.any.tensor_tensor(
                        out=oh[:].rearrange("p (j m) -> p j m", j=J),
                        in0=seg_sb[:, t * J:(t + 1) * J].to_broadcast(
                            [128, J, 128]),
                        in1=iota[:].rearrange("p (j m) -> p j m", j=J),
                        op=mybir.AluOpType.is_equal,
                    )
                    ps = ppool.tile([128, 128], mybir.dt.float32,
                                    space="PSUM", name="ps")
                    for j in range(J):
                        nc.tensor.matmul(
                            out=ps[:],
                            lhsT=oh[:, j * 128:(j + 1) * 128],
                            rhs=gb[:, j * 128:(j + 1) * 128],
                            start=(j == 0),
                            stop=(j == J - 1),
                        )
                    nc.any.tensor_copy(out=stg[:, tt * 128:(tt + 1) * 128],
                                       in_=ps[:])
                # staging -> DRAM: row (g*128 + p) <- stg[p, g*128:]
                out_view = out_p[t0 * 128:(t0 + used) * 128, :].rearrange(
                    "(g p) d -> p g d", p=128)
                nc.sync.dma_start(
                    out=out_view,
                    in_=stg[:, :used * 128].rearrange("p (g d) -> p g d",
                                                      d=128))
    nc.compile()
    return nc


def kernel(raw_h, member_idx, segment_ids):
    global LAST_EXEC_NS
    from concourse.bass_utils import run_bass_kernel_spmd

    if os.environ.get("BASS_TRACE"):
        _install_ntff_hook()

    raw = np.ascontiguousarray(np.asarray(raw_h, dtype=np.float32))
    J, idx_list, seg_list = _host_prep(member_idx, segment_ids)
    nc = _build_program(J)

    in_maps = [
        {"raw_h": raw, "idx_t": idx_list[c], "seg_t": seg_list[c]}
        for c in range(N_CORES)
    ]
    res = run_bass_kernel_spmd(nc, in_maps, core_ids=list(range(N_CORES)))
    LAST_EXEC_NS = res.exec_time_ns

    out = np.concatenate(
        [res.results[c]["out"][:CPC] for c in range(N_CORES)], axis=0)
    return out.astype(np.float32)
